# revision 3
# baseline (speedup 1.0000x reference)
"""Bayesian linear layer (mean-field reparameterization) on 8 TRN2 NeuronCores.

v9: 2D sharding (4 batch-blocks x 2 out-blocks) on top of the v5 design.
Each core handles 64 batches x 512 outputs: eps traffic is unchanged
(128 MiB/core) but the replicated psi/mu parameter load halves from
8 MB to 4 MB per core => ~11 us less HBM stream time per execution.
Kernel structure is otherwise identical to v5 (short-tail epilogue:
bt pre-scattered into staging, per-batch DVE add, incremental stores).
"""

import numpy as np

import os

BS, FOUT, IN = 256, 1024, 1024
NCORES = 8
NOB = 2                     # out blocks
NBB = NCORES // NOB         # batch blocks
BPC = BS // NBB             # 64 batches per core
OUT = FOUT // NOB           # 512 outputs per core
ICH = IN // 128             # 8 i-chunks
OH = max(1, OUT // 512)     # matmul output halves (1 here)
CPT_DEFAULT = int(os.environ.get("BK_CPT", "4"))
EBUFS_DEFAULT = int(os.environ.get("BK_EBUFS", "7"))
PBUFS_DEFAULT = int(os.environ.get("BK_PBUFS", "4"))

_cache = {}


def _build(reps, pe_mode, loop=False, cpt=None, ebufs=None, pbufs=None):
    CPT = cpt or CPT_DEFAULT
    EBUFS = ebufs or EBUFS_DEFAULT
    PBUFS = pbufs or PBUFS_DEFAULT
    import concourse.bass as bass
    import concourse.mybir as mybir
    import concourse.bacc as bacc
    from concourse import tile

    f32 = mybir.dt.float32
    f32r = mybir.dt.float32r
    bf16 = mybir.dt.bfloat16
    mult = mybir.AluOpType.mult
    add = mybir.AluOpType.add

    nc = bacc.Bacc(None, target_bir_lowering=False)

    d_eps = nc.dram_tensor("epsT", [BPC, IN, OUT], f32, kind="ExternalInput")
    d_xT = nc.dram_tensor("xT", [IN, BPC], f32, kind="ExternalInput")
    d_psiT = nc.dram_tensor("psiT", [IN, OUT], f32, kind="ExternalInput")
    d_muT = nc.dram_tensor("muT", [IN, OUT], f32, kind="ExternalInput")
    d_eb = nc.dram_tensor("eps_b", [BPC, OUT], f32, kind="ExternalInput")
    d_bpsi = nc.dram_tensor("bpsi", [1, OUT], f32, kind="ExternalInput")
    d_bmu = nc.dram_tensor("bmu", [1, OUT], f32, kind="ExternalInput")
    if loop:
        d_it = nc.dram_tensor("iters", [1, 1], mybir.dt.int32,
                              kind="ExternalInput")
    d_out = nc.dram_tensor("out", [BPC, OUT], f32, kind="ExternalOutput")

    with tile.TileContext(nc) as tc:
        with tc.tile_pool(name="const", bufs=1) as cpool, \
             tc.tile_pool(name="dbuf", bufs=2) as dpool, \
             tc.tile_pool(name="eps", bufs=EBUFS) as epool, \
             tc.tile_pool(name="p2b", bufs=PBUFS) as p2pool, \
             tc.tile_pool(name="ps", bufs=2, space="PSUM") as pspool, \
             tc.tile_pool(name="pst2", bufs=2, space="PSUM") as t2pool:

            p2dt = {"f32r": f32r, "f32": f32, "bf16": bf16}[pe_mode]
            sT = cpool.tile([128, ICH, OUT], f32, name="sT")
            xTf = cpool.tile([128, ICH, BPC], f32, name="xTf")
            xTm = cpool.tile([128, ICH, BPC], p2dt, name="xTm")
            ebt = cpool.tile([BPC, OUT], f32, name="ebt")
            sbrow = cpool.tile([1, OUT], f32, name="sbrow")
            sb_bc = cpool.tile([BPC, OUT], f32, name="sb_bc")
            mu_bc = cpool.tile([BPC, OUT], f32, name="mu_bc")
            bias_rows = cpool.tile([BPC, OUT], f32, name="bias_rows")
            staging = cpool.tile([128, BPC // 4, OUT], f32, name="staging")
            murow = cpool.tile([1, OUT], f32, name="murow")

            def emit(rep):
                # ---- prologue: params, exp(psi), bias rows, mu-term ----
                for ic in range(ICH):
                    pt = epool.tile([128, CPT, OUT], f32,
                                    name=f"psi_{rep}_{ic}", tag="eps")
                    nc.sync.dma_start(out=pt[:, 0, :],
                                      in_=d_psiT[ic * 128:(ic + 1) * 128, :])
                    nc.scalar.activation(sT[:, ic, :], pt[:, 0, :],
                                         mybir.ActivationFunctionType.Exp)
                    nc.sync.dma_start(out=xTf[:, ic, :],
                                      in_=d_xT[ic * 128:(ic + 1) * 128, :])
                nc.vector.tensor_copy(xTm[:], xTf[:])

                nc.sync.dma_start(out=ebt[:], in_=d_eb[:])
                nc.sync.dma_start(out=sbrow[:], in_=d_bpsi[:])
                nc.scalar.activation(sbrow[:], sbrow[:],
                                     mybir.ActivationFunctionType.Exp)
                nc.gpsimd.partition_broadcast(sb_bc[:], sbrow[:])
                nc.sync.dma_start(out=murow[:], in_=d_bmu[:])
                nc.gpsimd.partition_broadcast(mu_bc[:], murow[:])
                nc.vector.tensor_tensor(bias_rows[:], ebt[:], sb_bc[:], mult)
                nc.vector.tensor_tensor(bias_rows[:], bias_rows[:], mu_bc[:],
                                        add)

                # mu-term: t2[b, o] = sum_i x[b, i] * mu[o, i], M=BPC fp32
                t2ps = t2pool.tile([BPC, OUT], f32, name=f"t2_{rep}", tag="t2")
                for t in range(ICH // CPT):
                    mt = epool.tile([128, CPT, OUT], f32,
                                    name=f"mu_{rep}_{t}", tag="eps")
                    nc.sync.dma_start(
                        out=mt[:],
                        in_=d_muT[t * CPT * 128:(t + 1) * CPT * 128, :]
                        .rearrange("(s p) o -> p s o", p=128))
                    for s in range(CPT):
                        ic = t * CPT + s
                        for h in range(OH):
                            nc.tensor.matmul(
                                t2ps[:, h * 512:(h + 1) * 512],
                                xTf[:, ic, :],
                                mt[:, s, h * 512:(h + 1) * 512],
                                start=(ic == 0), stop=(ic == ICH - 1))

                # bt = mu-term + bias rows; pre-scatter into staging slots
                bt = dpool.tile([BPC, OUT], f32, name=f"bt_{rep}", tag="bt")
                nc.vector.tensor_tensor(bt[:], t2ps[:], bias_rows[:], add)
                for g in range(BPC // 4):
                    nc.scalar.dma_start(out=staging[0:128:32, g, :],
                                        in_=bt[4 * g:4 * g + 4, :])

                # ---- main loop: eps-term matvecs ----
                for b in range(BPC):
                    ps = pspool.tile([1, OUT], f32, name=f"ps_{rep}_{b}",
                                     tag="ps")
                    for t in range(ICH // CPT):
                        e = epool.tile([128, CPT, OUT], f32,
                                       name=f"e_{rep}_{b}_{t}", tag="eps")
                        nc.sync.dma_start(
                            out=e[:],
                            in_=d_eps[b, t * CPT * 128:(t + 1) * CPT * 128, :]
                            .rearrange("(s p) o -> p s o", p=128))
                        p2 = p2pool.tile([128, CPT, OUT], p2dt,
                                         name=f"p2_{rep}_{b}_{t}", tag="p2")
                        nc.vector.tensor_tensor(
                            p2[:], e[:], sT[:, t * CPT:(t + 1) * CPT, :], mult)
                        for s in range(CPT):
                            ic = t * CPT + s
                            for h in range(OH):
                                nc.tensor.matmul(
                                    ps[:, h * 512:(h + 1) * 512],
                                    xTm[:, ic, b:b + 1],
                                    p2[:, s, h * 512:(h + 1) * 512],
                                    start=(ic == 0), stop=(ic == ICH - 1))
                    # finalize row in place: staging[32j, g] += ps
                    j, g = b % 4, b // 4
                    nc.vector.tensor_tensor(
                        staging[32 * j:32 * j + 1, g, :],
                        staging[32 * j:32 * j + 1, g, :],
                        ps[:], add)
                    if j == 3:
                        # group g complete: store it now (ACT ring)
                        nc.scalar.dma_start(
                            out=d_out[4 * g:4 * g + 4, :],
                            in_=staging[0:128:32, g, :])

            if loop:
                it_sb = cpool.tile([1, 1], mybir.dt.int32, name="it_sb")
                nc.sync.dma_start(out=it_sb[:], in_=d_it[:])
                regs = []
                for et in mybir.ALL_ENGINES:
                    eng = nc.engines[et]
                    r = eng.alloc_register(f"iters_{et.name}")
                    eng.reg_load(r, it_sb[0:1, 0:1])
                    regs.append(r)
                iters_val = bass.make_scalar_value(
                    bass.RegisterHandles(regs), min_val=1, max_val=1 << 20)
                with tc.For_i(0, iters_val, 1,
                              hint_engines=(mybir.EngineType.PE,
                                            mybir.EngineType.DVE,
                                            mybir.EngineType.SP)):
                    emit(0)
            else:
                for rep in range(reps):
                    emit(rep)

    nc.compile()
    return nc


def _get_nc(reps, pe_mode, loop=False, cpt=None, ebufs=None, pbufs=None):
    key = (reps, pe_mode, loop, cpt, ebufs, pbufs)
    if key not in _cache:
        _cache[key] = _build(reps, pe_mode, loop, cpt, ebufs, pbufs)
    return _cache[key]


def _prepare_inmaps(x, weight_mu, weight_psi, bias_mu, bias_psi, eps_w, eps_b):
    x = np.asarray(x, dtype=np.float32)
    weight_mu = np.asarray(weight_mu, dtype=np.float32)
    weight_psi = np.asarray(weight_psi, dtype=np.float32)
    bias_mu = np.asarray(bias_mu, dtype=np.float32)
    bias_psi = np.asarray(bias_psi, dtype=np.float32)
    eps_w = np.asarray(eps_w, dtype=np.float32)
    eps_b = np.asarray(eps_b, dtype=np.float32)

    in_maps = []
    for c in range(NCORES):
        bc, oc = divmod(c, NOB)
        slb = slice(bc * BPC, (bc + 1) * BPC)
        slo = slice(oc * OUT, (oc + 1) * OUT)
        in_maps.append({
            "epsT": np.ascontiguousarray(
                eps_w[slb, slo, :].transpose(0, 2, 1)),
            "xT": np.ascontiguousarray(x[slb].T),
            "psiT": np.ascontiguousarray(weight_psi[slo].T),
            "muT": np.ascontiguousarray(weight_mu[slo].T),
            "eps_b": np.ascontiguousarray(eps_b[slb, slo]),
            "bpsi": np.ascontiguousarray(bias_psi[slo].reshape(1, OUT)),
            "bmu": np.ascontiguousarray(bias_mu[slo].reshape(1, OUT)),
        })
    return in_maps


def _assemble(outs):
    """outs: per-core [BPC, OUT] arrays, indexable by core id."""
    full = np.empty((BS, FOUT), np.float32)
    for c in range(NCORES):
        bc, oc = divmod(c, NOB)
        full[bc * BPC:(bc + 1) * BPC, oc * OUT:(oc + 1) * OUT] = outs[c]
    return full


def _run(in_maps, reps=1, pe_mode="f32r", loop_iters=None, cpt=None,
         ebufs=None, pbufs=None):
    from concourse.bass_utils import run_bass_kernel_spmd
    nc = _get_nc(reps, pe_mode, loop=loop_iters is not None, cpt=cpt,
                 ebufs=ebufs, pbufs=pbufs)
    if loop_iters is not None:
        it = np.array([[loop_iters]], dtype=np.int32)
        in_maps = [{**m, "iters": it} for m in in_maps]
    res = run_bass_kernel_spmd(nc, in_maps, core_ids=list(range(NCORES)))
    return _assemble([res.results[c]["out"] for c in range(NCORES)])


def kernel(x, weight_mu, weight_psi, bias_mu, bias_psi, eps_w, eps_b,
           _pe_mode="f32r"):
    in_maps = _prepare_inmaps(x, weight_mu, weight_psi, bias_mu, bias_psi,
                              eps_w, eps_b)
    try:
        return _run(in_maps, pe_mode=_pe_mode)
    except Exception:
        if _pe_mode == "f32":
            raise
        # fall back to plain-fp32 PE path (4 cyc/row, bit-safer numerics)
        return _run(in_maps, pe_mode="f32")


# revision 4
# speedup vs baseline: 1.0077x; 1.0077x over previous
"""Bayesian linear layer (mean-field reparameterization) on 8 TRN2 NeuronCores.

v9: 2D sharding (4 batch-blocks x 2 out-blocks) on top of the v5 design.
Each core handles 64 batches x 512 outputs: eps traffic is unchanged
(128 MiB/core) but the replicated psi/mu parameter load halves from
8 MB to 4 MB per core => ~11 us less HBM stream time per execution.
Kernel structure is otherwise identical to v5 (short-tail epilogue:
bt pre-scattered into staging, per-batch DVE add, incremental stores).
"""

import numpy as np

import os

BS, FOUT, IN = 256, 1024, 1024
NCORES = 8
NOB = 4                     # out blocks
NBB = NCORES // NOB         # batch blocks
BPC = BS // NBB             # 64 batches per core
OUT = FOUT // NOB           # 512 outputs per core
ICH = IN // 128             # 8 i-chunks
OH = max(1, OUT // 512)     # matmul output halves (1 here)
HW = min(512, OUT)          # matmul output width
CPT_DEFAULT = int(os.environ.get("BK_CPT", "8"))
EBUFS_DEFAULT = int(os.environ.get("BK_EBUFS", "7"))
PBUFS_DEFAULT = int(os.environ.get("BK_PBUFS", "4"))

_cache = {}


def _build(reps, pe_mode, loop=False, cpt=None, ebufs=None, pbufs=None):
    CPT = cpt or CPT_DEFAULT
    EBUFS = ebufs or EBUFS_DEFAULT
    PBUFS = pbufs or PBUFS_DEFAULT
    import concourse.bass as bass
    import concourse.mybir as mybir
    import concourse.bacc as bacc
    from concourse import tile

    f32 = mybir.dt.float32
    f32r = mybir.dt.float32r
    bf16 = mybir.dt.bfloat16
    mult = mybir.AluOpType.mult
    add = mybir.AluOpType.add

    nc = bacc.Bacc(None, target_bir_lowering=False)

    d_eps = nc.dram_tensor("epsT", [BPC, IN, OUT], f32, kind="ExternalInput")
    d_xT = nc.dram_tensor("xT", [IN, BPC], f32, kind="ExternalInput")
    d_psiT = nc.dram_tensor("psiT", [IN, OUT], f32, kind="ExternalInput")
    d_muT = nc.dram_tensor("muT", [IN, OUT], f32, kind="ExternalInput")
    d_eb = nc.dram_tensor("eps_b", [BPC, OUT], f32, kind="ExternalInput")
    d_bpsi = nc.dram_tensor("bpsi", [1, OUT], f32, kind="ExternalInput")
    d_bmu = nc.dram_tensor("bmu", [1, OUT], f32, kind="ExternalInput")
    if loop:
        d_it = nc.dram_tensor("iters", [1, 1], mybir.dt.int32,
                              kind="ExternalInput")
    d_out = nc.dram_tensor("out", [BPC, OUT], f32, kind="ExternalOutput")

    with tile.TileContext(nc) as tc:
        with tc.tile_pool(name="const", bufs=1) as cpool, \
             tc.tile_pool(name="dbuf", bufs=2) as dpool, \
             tc.tile_pool(name="eps", bufs=EBUFS) as epool, \
             tc.tile_pool(name="p2b", bufs=PBUFS) as p2pool, \
             tc.tile_pool(name="ps", bufs=2, space="PSUM") as pspool, \
             tc.tile_pool(name="pst2", bufs=2, space="PSUM") as t2pool:

            p2dt = {"f32r": f32r, "f32": f32, "bf16": bf16}[pe_mode]
            sT = cpool.tile([128, ICH, OUT], f32, name="sT")
            xTf = cpool.tile([128, ICH, BPC], f32, name="xTf")
            xTm = cpool.tile([128, ICH, BPC], p2dt, name="xTm")
            ebt = cpool.tile([BPC, OUT], f32, name="ebt")
            sbrow = cpool.tile([1, OUT], f32, name="sbrow")
            sb_bc = cpool.tile([BPC, OUT], f32, name="sb_bc")
            mu_bc = cpool.tile([BPC, OUT], f32, name="mu_bc")
            bias_rows = cpool.tile([BPC, OUT], f32, name="bias_rows")
            staging = cpool.tile([128, BPC // 4, OUT], f32, name="staging")
            murow = cpool.tile([1, OUT], f32, name="murow")

            def emit(rep):
                # ---- prologue: params, exp(psi), bias rows, mu-term ----
                for ic in range(ICH):
                    pt = epool.tile([128, CPT, OUT], f32,
                                    name=f"psi_{rep}_{ic}", tag="eps")
                    nc.sync.dma_start(out=pt[:, 0, :],
                                      in_=d_psiT[ic * 128:(ic + 1) * 128, :])
                    nc.scalar.activation(sT[:, ic, :], pt[:, 0, :],
                                         mybir.ActivationFunctionType.Exp)
                    nc.sync.dma_start(out=xTf[:, ic, :],
                                      in_=d_xT[ic * 128:(ic + 1) * 128, :])
                nc.vector.tensor_copy(xTm[:], xTf[:])

                nc.sync.dma_start(out=ebt[:], in_=d_eb[:])
                nc.sync.dma_start(out=sbrow[:], in_=d_bpsi[:])
                nc.scalar.activation(sbrow[:], sbrow[:],
                                     mybir.ActivationFunctionType.Exp)
                nc.gpsimd.partition_broadcast(sb_bc[:], sbrow[:])
                nc.sync.dma_start(out=murow[:], in_=d_bmu[:])
                nc.gpsimd.partition_broadcast(mu_bc[:], murow[:])
                nc.vector.tensor_tensor(bias_rows[:], ebt[:], sb_bc[:], mult)
                nc.vector.tensor_tensor(bias_rows[:], bias_rows[:], mu_bc[:],
                                        add)

                # mu-term: t2[b, o] = sum_i x[b, i] * mu[o, i], M=BPC fp32
                t2ps = t2pool.tile([BPC, OUT], f32, name=f"t2_{rep}", tag="t2")
                for t in range(ICH // CPT):
                    mt = epool.tile([128, CPT, OUT], f32,
                                    name=f"mu_{rep}_{t}", tag="eps")
                    nc.sync.dma_start(
                        out=mt[:],
                        in_=d_muT[t * CPT * 128:(t + 1) * CPT * 128, :]
                        .rearrange("(s p) o -> p s o", p=128))
                    for s in range(CPT):
                        ic = t * CPT + s
                        for h in range(OH):
                            nc.tensor.matmul(
                                t2ps[:, h * HW:(h + 1) * HW],
                                xTf[:, ic, :],
                                mt[:, s, h * HW:(h + 1) * HW],
                                start=(ic == 0), stop=(ic == ICH - 1))

                # bt = mu-term + bias rows; pre-scatter into staging slots
                bt = dpool.tile([BPC, OUT], f32, name=f"bt_{rep}", tag="bt")
                nc.vector.tensor_tensor(bt[:], t2ps[:], bias_rows[:], add)
                for g in range(BPC // 4):
                    nc.scalar.dma_start(out=staging[0:128:32, g, :],
                                        in_=bt[4 * g:4 * g + 4, :])

                # ---- main loop: eps-term matvecs ----
                for b in range(BPC):
                    ps = pspool.tile([1, OUT], f32, name=f"ps_{rep}_{b}",
                                     tag="ps")
                    for t in range(ICH // CPT):
                        e = epool.tile([128, CPT, OUT], f32,
                                       name=f"e_{rep}_{b}_{t}", tag="eps")
                        nc.sync.dma_start(
                            out=e[:],
                            in_=d_eps[b, t * CPT * 128:(t + 1) * CPT * 128, :]
                            .rearrange("(s p) o -> p s o", p=128))
                        p2 = p2pool.tile([128, CPT, OUT], p2dt,
                                         name=f"p2_{rep}_{b}_{t}", tag="p2")
                        nc.vector.tensor_tensor(
                            p2[:], e[:], sT[:, t * CPT:(t + 1) * CPT, :], mult)
                        for s in range(CPT):
                            ic = t * CPT + s
                            for h in range(OH):
                                nc.tensor.matmul(
                                    ps[:, h * HW:(h + 1) * HW],
                                    xTm[:, ic, b:b + 1],
                                    p2[:, s, h * HW:(h + 1) * HW],
                                    start=(ic == 0), stop=(ic == ICH - 1))
                    # finalize row in place: staging[32j, g] += ps
                    j, g = b % 4, b // 4
                    nc.vector.tensor_tensor(
                        staging[32 * j:32 * j + 1, g, :],
                        staging[32 * j:32 * j + 1, g, :],
                        ps[:], add)
                    if j == 3:
                        # group g complete: store it now (ACT ring)
                        nc.scalar.dma_start(
                            out=d_out[4 * g:4 * g + 4, :],
                            in_=staging[0:128:32, g, :])

            if loop:
                it_sb = cpool.tile([1, 1], mybir.dt.int32, name="it_sb")
                nc.sync.dma_start(out=it_sb[:], in_=d_it[:])
                regs = []
                for et in mybir.ALL_ENGINES:
                    eng = nc.engines[et]
                    r = eng.alloc_register(f"iters_{et.name}")
                    eng.reg_load(r, it_sb[0:1, 0:1])
                    regs.append(r)
                iters_val = bass.make_scalar_value(
                    bass.RegisterHandles(regs), min_val=1, max_val=1 << 20)
                with tc.For_i(0, iters_val, 1,
                              hint_engines=(mybir.EngineType.PE,
                                            mybir.EngineType.DVE,
                                            mybir.EngineType.SP)):
                    emit(0)
            else:
                for rep in range(reps):
                    emit(rep)

    nc.compile()
    return nc


def _get_nc(reps, pe_mode, loop=False, cpt=None, ebufs=None, pbufs=None):
    key = (reps, pe_mode, loop, cpt, ebufs, pbufs)
    if key not in _cache:
        _cache[key] = _build(reps, pe_mode, loop, cpt, ebufs, pbufs)
    return _cache[key]


def _prepare_inmaps(x, weight_mu, weight_psi, bias_mu, bias_psi, eps_w, eps_b):
    x = np.asarray(x, dtype=np.float32)
    weight_mu = np.asarray(weight_mu, dtype=np.float32)
    weight_psi = np.asarray(weight_psi, dtype=np.float32)
    bias_mu = np.asarray(bias_mu, dtype=np.float32)
    bias_psi = np.asarray(bias_psi, dtype=np.float32)
    eps_w = np.asarray(eps_w, dtype=np.float32)
    eps_b = np.asarray(eps_b, dtype=np.float32)

    in_maps = []
    for c in range(NCORES):
        bc, oc = divmod(c, NOB)
        slb = slice(bc * BPC, (bc + 1) * BPC)
        slo = slice(oc * OUT, (oc + 1) * OUT)
        in_maps.append({
            "epsT": np.ascontiguousarray(
                eps_w[slb, slo, :].transpose(0, 2, 1)),
            "xT": np.ascontiguousarray(x[slb].T),
            "psiT": np.ascontiguousarray(weight_psi[slo].T),
            "muT": np.ascontiguousarray(weight_mu[slo].T),
            "eps_b": np.ascontiguousarray(eps_b[slb, slo]),
            "bpsi": np.ascontiguousarray(bias_psi[slo].reshape(1, OUT)),
            "bmu": np.ascontiguousarray(bias_mu[slo].reshape(1, OUT)),
        })
    return in_maps


def _assemble(outs):
    """outs: per-core [BPC, OUT] arrays, indexable by core id."""
    full = np.empty((BS, FOUT), np.float32)
    for c in range(NCORES):
        bc, oc = divmod(c, NOB)
        full[bc * BPC:(bc + 1) * BPC, oc * OUT:(oc + 1) * OUT] = outs[c]
    return full


def _run(in_maps, reps=1, pe_mode="f32r", loop_iters=None, cpt=None,
         ebufs=None, pbufs=None):
    from concourse.bass_utils import run_bass_kernel_spmd
    nc = _get_nc(reps, pe_mode, loop=loop_iters is not None, cpt=cpt,
                 ebufs=ebufs, pbufs=pbufs)
    if loop_iters is not None:
        it = np.array([[loop_iters]], dtype=np.int32)
        in_maps = [{**m, "iters": it} for m in in_maps]
    res = run_bass_kernel_spmd(nc, in_maps, core_ids=list(range(NCORES)))
    return _assemble([res.results[c]["out"] for c in range(NCORES)])


def kernel(x, weight_mu, weight_psi, bias_mu, bias_psi, eps_w, eps_b,
           _pe_mode="f32r"):
    in_maps = _prepare_inmaps(x, weight_mu, weight_psi, bias_mu, bias_psi,
                              eps_w, eps_b)
    try:
        return _run(in_maps, pe_mode=_pe_mode)
    except Exception:
        if _pe_mode == "f32":
            raise
        # fall back to plain-fp32 PE path (4 cyc/row, bit-safer numerics)
        return _run(in_maps, pe_mode="f32")
